# revision 8
# baseline (speedup 1.0000x reference)
"""Chunkwise causal attention (B=2, S=4096, H=16, D=64, CHUNK=128) on 8 TRN2 NeuronCores.

Sharding: head-parallel tensor parallelism. Core c owns heads (2c, 2c+1) for both
batches: it computes the qkv projection for its heads (w_qkv column slice), full
causal attention for its 4 (batch, head) units, and a partial out-projection
(w_out row slice). Host sums the 8 partial outputs.

Device kernel layout notes:
 - x is passed host-transposed as xT [1024, 8192] bf16 so the qkv contraction dim
   (hidden) lands on SBUF partitions without any device-side transpose.
 - Q^T, K^T are kept head-major [128 = 2 heads x 64, S]; V is kept key-major
   [keys, 2, 65] with a ones column so the P@V matmul also produces the softmax
   denominator (row 64 of the PV psum).
 - Scores are computed transposed (scores^T [keys, queries]) so the exp'd
   probabilities are directly the moving operand of the P@V matmul - no
   transposes anywhere on the device.
 - Softmax skips max-subtraction (scores ~ N(0,1): exp never overflows in f32);
   causal masking multiplies the exp'd diagonal blocks by a precomputed 0/1 mask.
"""

import sys

if "/opt/trn_rl_repo" not in sys.path:
    sys.path.insert(0, "/opt/trn_rl_repo")

import numpy as np
import ml_dtypes

B = 2
S = 4096
HID = 1024
NHEAD = 16
D = 64
CH = 128  # key chunk (= reference CHUNK)
G = 512  # query group (4 chunks)
NGB = S // G  # 8 query groups per batch
KK = HID // 128  # 8 contraction chunks for the projections
NKC = S // CH  # 32 key chunks per batch
TT = B * S  # 8192 tokens across batches

_CACHE = {}


def _build_nc():
    import concourse.mybir as mybir
    import concourse.tile as tile
    from concourse import bacc
    from contextlib import ExitStack

    bf16 = mybir.dt.bfloat16
    f32 = mybir.dt.float32
    Exp = mybir.ActivationFunctionType.Exp
    mult = mybir.AluOpType.mult

    nc = bacc.Bacc("TRN2", target_bir_lowering=False, debug=False)
    xT_d = nc.dram_tensor("xT", [HID, TT], bf16, kind="ExternalInput")
    wq_d = nc.dram_tensor("wq", [HID, 128], bf16, kind="ExternalInput")
    wk_d = nc.dram_tensor("wk", [HID, 128], bf16, kind="ExternalInput")
    wv_d = nc.dram_tensor("wv", [HID, 128], bf16, kind="ExternalInput")
    wo_d = nc.dram_tensor("wo", [128, HID], bf16, kind="ExternalInput")
    mask_d = nc.dram_tensor("mask", [128, 4 * G], bf16, kind="ExternalInput")
    out_d = nc.dram_tensor("out", [TT, HID], f32, kind="ExternalOutput")

    xT_r = xT_d.rearrange("(kk p) t -> p kk t", p=128)
    wq_r = wq_d.rearrange("(kk p) c -> p kk c", p=128)
    wk_r = wk_d.rearrange("(kk p) c -> p kk c", p=128)
    wv_r = wv_d.rearrange("(kk p) c -> p kk c", p=128)

    with tile.TileContext(nc) as tc, ExitStack() as ctx:
        consts = ctx.enter_context(tc.tile_pool(name="consts", bufs=1))
        qkv_pool = ctx.enter_context(tc.tile_pool(name="qkv", bufs=2))
        xt_pool = ctx.enter_context(tc.tile_pool(name="xt", bufs=3))
        exp_pool = ctx.enter_context(tc.tile_pool(name="exp", bufs=4))
        attn_pool = ctx.enter_context(tc.tile_pool(name="attn", bufs=4))
        norm_pool = ctx.enter_context(tc.tile_pool(name="norm", bufs=3))
        osb_pool = ctx.enter_context(tc.tile_pool(name="osb", bufs=3))
        ps_mm = ctx.enter_context(tc.tile_pool(name="psmm", bufs=2, space="PSUM"))
        ps_sq = ctx.enter_context(tc.tile_pool(name="pssq", bufs=2, space="PSUM"))
        ps_pv = ctx.enter_context(tc.tile_pool(name="pspv", bufs=2, space="PSUM"))

        wq_sb = consts.tile([128, KK, 128], bf16, tag="wq")
        wk_sb = consts.tile([128, KK, 128], bf16, tag="wk")
        wv_sb = consts.tile([128, KK, 128], bf16, tag="wv")
        wo_sb = consts.tile([128, HID], bf16, tag="wo")
        mask_sb = consts.tile([128, 4 * G], bf16, tag="mask")
        ones_sb = consts.tile([1, 64], bf16, tag="ones")
        nc.sync.dma_start(wq_sb[:], wq_r)
        nc.sync.dma_start(wk_sb[:], wk_r)
        nc.sync.dma_start(wv_sb[:], wv_r)
        nc.sync.dma_start(wo_sb[:], wo_d[:])
        nc.sync.dma_start(mask_sb[:], mask_d[:])
        nc.vector.memset(ones_sb[:], 1.0)

        for b in range(B):
            QT = qkv_pool.tile([128, S], bf16, tag="QT")
            KT = qkv_pool.tile([128, S], bf16, tag="KT")
            V = qkv_pool.tile([128, NKC, 2, 65], bf16, tag="V")
            nc.gpsimd.memset(V[:, :, :, 64:65], 1.0)

            for g in range(NGB):
                t0 = b * S + g * G

                # ---- phase 1: qkv projection for this token group ----
                xt = xt_pool.tile([128, KK, G], bf16, tag="xt")
                nc.sync.dma_start(xt[:], xT_r[:, :, t0 : t0 + G])
                for w_sb, dstT in ((wq_sb, QT), (wk_sb, KT)):
                    ps = ps_mm.tile([128, G], f32, tag="mm")
                    for kk in range(KK):
                        nc.tensor.matmul(
                            ps[:],
                            w_sb[:, kk, :],
                            xt[:, kk, :],
                            start=(kk == 0),
                            stop=(kk == KK - 1),
                        )
                    nc.vector.tensor_copy(dstT[:, g * G : (g + 1) * G], ps[:])
                for tch in range(G // CH):
                    ps = ps_mm.tile([128, G], f32, tag="mm")
                    for kk in range(KK):
                        nc.tensor.matmul(
                            ps[:, 0:CH],
                            xt[:, kk, tch * CH : (tch + 1) * CH],
                            wv_sb[:, kk, :],
                            start=(kk == 0),
                            stop=(kk == KK - 1),
                        )
                    kc = g * 4 + tch
                    nc.vector.tensor_copy(
                        V[:, kc, :, 0:64],
                        ps[:, 0:CH].rearrange("p (h d) -> p h d", h=2),
                    )

                # ---- phase 2: attention for query group g (keys 0..4g+3) ----
                nkc = 4 * g + 4  # causal key chunks for this group
                pv = [ps_pv.tile([65, G], f32, tag="pv", name=f"pv{h}") for h in range(2)]
                for qd in range(nkc // 2):
                    sq = [ps_sq.tile([128, 2 * G], f32, tag="sq", name=f"sq{h}") for h in range(2)]
                    for j in range(2):
                        kc = qd * 2 + j
                        for h in range(2):
                            nc.tensor.matmul(
                                sq[h][:, j * G : (j + 1) * G],
                                KT[h * 64 : (h + 1) * 64, kc * CH : (kc + 1) * CH],
                                QT[h * 64 : (h + 1) * 64, g * G : (g + 1) * G],
                            )
                    ex = [exp_pool.tile([128, 2 * G], bf16, tag="exp", name=f"ex{h}") for h in range(2)]
                    for h in range(2):
                        # exp(scores / sqrt(D)); scale folded into the ACT affine
                        nc.scalar.activation(ex[h][:], sq[h][:], Exp, scale=0.125)
                    if qd >= nkc // 2 - 2:
                        seg = qd - (nkc // 2 - 2)  # 0 -> j 0/1 masks, 1 -> j 2/3
                        for h in range(2):
                            nc.vector.tensor_tensor(
                                ex[h][:],
                                ex[h][:],
                                mask_sb[:, seg * 2 * G : (seg + 1) * 2 * G],
                                op=mult,
                            )
                    for j in range(2):
                        kc = qd * 2 + j
                        for h in range(2):
                            nc.tensor.matmul(
                                pv[h][:],
                                V[:, kc, h, :],
                                ex[h][:, j * G : (j + 1) * G],
                                start=(kc == 0),
                                stop=(kc == nkc - 1),
                            )

                # ---- normalize (unnormalized PV x broadcast reciprocal) ----
                den = norm_pool.tile([1, 2 * G], f32, tag="den")
                nc.vector.tensor_copy(den[0:1, 0:G], pv[0][64:65, :])
                nc.vector.tensor_copy(den[0:1, G : 2 * G], pv[1][64:65, :])
                rec = norm_pool.tile([1, 2 * G], bf16, tag="rec")
                with nc.allow_low_precision(reason="softmax denominator reciprocal in bf16"):
                    nc.vector.reciprocal(rec[:], den[:])
                bcp = ps_mm.tile([128, G], f32, tag="mm")
                nc.tensor.matmul(bcp[0:64, :], ones_sb[:], rec[0:1, 0:G])
                nc.tensor.matmul(
                    bcp[64:128, :], ones_sb[:], rec[0:1, G : 2 * G], tile_position=(0, 64)
                )
                bcs = norm_pool.tile([128, G], f32, tag="bcs")
                nc.vector.tensor_copy(bcs[:], bcp[:])
                at = attn_pool.tile([128, G], bf16, tag="attnT")
                nc.vector.tensor_tensor(at[0:64, :], pv[0][0:64, :], bcs[0:64, :], op=mult)
                nc.vector.tensor_tensor(at[64:128, :], pv[1][0:64, :], bcs[64:128, :], op=mult)

                # ---- phase 3: partial out-projection for these 512 tokens ----
                for tch in range(G // CH):
                    for nn in range(2):
                        pso = ps_mm.tile([128, G], f32, tag="mm")
                        nc.tensor.matmul(
                            pso[:],
                            at[:, tch * CH : (tch + 1) * CH],
                            wo_sb[:, nn * G : (nn + 1) * G],
                        )
                        ob = osb_pool.tile([128, G], f32, tag="ob")
                        nc.vector.tensor_copy(ob[:], pso[:])
                        nc.sync.dma_start(
                            out_d[t0 + tch * CH : t0 + (tch + 1) * CH, nn * G : (nn + 1) * G],
                            ob[:],
                        )
    nc.compile()
    return nc


def _causal_mask():
    # mask[k, j*G + q] = 1 if query q (within the 512-query group) attends to
    # key k of diagonal-band chunk j, i.e. q >= j*128 + k.
    q = np.arange(G)
    k = np.arange(128)
    m = np.zeros((128, 4 * G), dtype=np.float32)
    for j in range(4):
        m[:, j * G : (j + 1) * G] = (q[None, :] >= (j * CH + k)[:, None]).astype(
            np.float32
        )
    return m.astype(ml_dtypes.bfloat16)


def get_nc():
    if "nc" not in _CACHE:
        _CACHE["nc"] = _build_nc()
    return _CACHE["nc"]


def make_in_maps(x, w_qkv, w_out):
    bf16 = ml_dtypes.bfloat16
    xf = np.asarray(x, dtype=np.float32).reshape(TT, HID)
    xT = np.ascontiguousarray(xf.T).astype(bf16)
    wqkv = np.asarray(w_qkv, dtype=np.float32)
    wout = np.asarray(w_out, dtype=np.float32)
    mask = _causal_mask()
    in_maps = []
    for c in range(8):
        c0 = 128 * c
        in_maps.append(
            {
                "xT": xT,
                "wq": np.ascontiguousarray(wqkv[:, c0 : c0 + 128]).astype(bf16),
                "wk": np.ascontiguousarray(wqkv[:, HID + c0 : HID + c0 + 128]).astype(bf16),
                "wv": np.ascontiguousarray(wqkv[:, 2 * HID + c0 : 2 * HID + c0 + 128]).astype(bf16),
                "wo": np.ascontiguousarray(wout[c0 : c0 + 128, :]).astype(bf16),
                "mask": mask,
            }
        )
    return in_maps


def kernel(x, w_qkv, w_out):
    from concourse.bass_utils import run_bass_kernel_spmd

    nc = get_nc()
    in_maps = make_in_maps(x, w_qkv, w_out)
    res = run_bass_kernel_spmd(nc, in_maps, core_ids=list(range(8)))
    acc = np.zeros((TT, HID), dtype=np.float32)
    for r in res.results:
        acc += r["out"]
    return acc.reshape(B, S, HID)
